# revision 18
# baseline (speedup 1.0000x reference)
"""Trainium2 Bass kernel for nn_Attn_69801808495303.

Computes, for encoder_outputs [L, B, 2H], W [H, 2H], b [H], v [H, 1]:
    energy = tanh(enc @ W.T + b)          # [L, B, H]
    scores = energy @ v                   # [L, B]
    attn   = softmax over B (per (L, f))  # broadcast over num_features
    out    = attn as [B, num_features, L]

Strategy: shard over L across 8 NeuronCores (embarrassingly parallel —
the softmax over batch is local to every L row). Host pre-transposes the
encoder shard to [2H, L_loc*B] bf16 so the contraction dim lands on SBUF
partitions; W/b/v are replicated. On device the TensorEngine runs only
the bf16 GEMM (W stationary, energy.T [h, m] tiles in PSUM); ScalarE
applies tanh+bias and the per-partition *v scale; VectorE accumulates the
8 h-tiles; GpSimd reduces over partitions to finish scores = v.tanh(...);
the 64-wide batch softmax runs in quarters so it hides under the GEMM.
Each core returns its [L_loc, B] probability block; the host concatenates
and broadcasts over num_features.
"""

import sys

for _p in ("/opt/trn_rl_repo", "/opt/pypackages"):
    if _p not in sys.path:
        sys.path.append(_p)

import numpy as np
import ml_dtypes

try:  # bass_utils imports this when BASS_TRACE is set; stub so tracing
    import antenv.axon_hooks  # noqa: F401  # degrades instead of crashing
except ImportError:
    import types

    _m = types.ModuleType("antenv.axon_hooks")
    _m._hook = None
    _m.set_axon_ntff_profile_hook = lambda h: setattr(_m, "_hook", h)
    _m.get_axon_ntff_profile_hook = lambda: _m._hook
    sys.modules["antenv.axon_hooks"] = _m

L, B, H, D = 2048, 64, 1024, 2048  # D = 2H
N_CORES = 8
L_LOC = L // N_CORES        # 256 rows of L per core
M = L_LOC * B               # 16384 tokens per core
M_BLK = 512
N_BLKS = M // M_BLK         # 32
D_TILES = D // 128          # 16
H_TILES = H // 128          # 8

BF16 = ml_dtypes.bfloat16

_compiled = {}
LAST_RESULTS = None


def _build():
    import concourse.mybir as mybir
    import concourse.tile as tile
    from concourse import bacc, bass_isa

    fp32, bf16 = mybir.dt.float32, mybir.dt.bfloat16
    AF = mybir.ActivationFunctionType

    nc = bacc.Bacc("TRN2", target_bir_lowering=False, debug=False,
                   num_devices=N_CORES)

    encT = nc.dram_tensor("encT", [D, M], bf16, kind="ExternalInput").ap()
    # weights pre-grouped by h-tile: wr[ht, d, j] = W[ht*128+j, d]
    wr = nc.dram_tensor("wr", [H_TILES, D, 128], bf16,
                        kind="ExternalInput").ap()
    bT = nc.dram_tensor("bT", [128, H_TILES], fp32, kind="ExternalInput").ap()
    vT = nc.dram_tensor("vT", [128, H_TILES], fp32, kind="ExternalInput").ap()
    out = nc.dram_tensor("out", [L_LOC, B], fp32, kind="ExternalOutput").ap()

    encT_t = encT.rearrange("(dt p) m -> p dt m", p=128)  # [128, D_TILES, M]
    wr_t = wr.rearrange("ht (dt p) j -> ht p dt j", p=128)

    with tile.TileContext(nc) as tc:
        with (
            tc.tile_pool(name="const", bufs=1) as cpool,
            tc.tile_pool(name="enc", bufs=32) as epool,
            tc.tile_pool(name="eng", bufs=4) as gpool,
            tc.tile_pool(name="veng", bufs=16) as vpool,
            tc.tile_pool(name="accp", bufs=3) as apool,
            tc.tile_pool(name="misc", bufs=2) as mpool,
            tc.tile_pool(name="psum_e", bufs=6, space="PSUM") as pe_pool,
            tc.tile_pool(name="psum_s", bufs=1, space="PSUM") as ps1pool,
            tc.tile_pool(name="dram", bufs=1, space="DRAM") as dpool,
        ):
            # Interleave the first et block's chunks with the weight DMAs so
            # the first matmuls start as soon as possible.
            wt_sb = [cpool.tile([128, D_TILES, 128], bf16, name=f"wt{ht}")
                     for ht in range(H_TILES)]

            def load_et(mb):
                msl = slice(mb * M_BLK, (mb + 1) * M_BLK)
                chunks = []
                for i in range(8):
                    ch = epool.tile([128, 2, M_BLK], bf16, tag="enc",
                                    name=f"et{mb}_{i}")
                    nc.sync.dma_start(ch[:], encT_t[:, 2 * i:2 * i + 2, msl])
                    chunks.append(ch)
                return chunks

            nc.sync.dma_start(wt_sb[0][:, 0:8, :], wr_t[0, :, 0:8, :])
            nc.sync.dma_start(wt_sb[0][:, 8:16, :], wr_t[0, :, 8:16, :])
            et0 = load_et(0)
            for ht in range(1, H_TILES):
                nc.sync.dma_start(wt_sb[ht][:, 0:8, :], wr_t[ht, :, 0:8, :])
                nc.sync.dma_start(wt_sb[ht][:, 8:16, :], wr_t[ht, :, 8:16, :])
            b_sb = cpool.tile([128, H_TILES], fp32)
            nc.sync.dma_start(b_sb[:], bT[:])
            v_sb = cpool.tile([128, H_TILES], fp32)
            nc.sync.dma_start(v_sb[:], vT[:])

            sc_dram = dpool.tile([1, M], fp32)

            # Warm the PE (HAM un-throttle needs ~3.4us of activity) while
            # the first weight/enc DMAs are in flight. The 4-byte DMA keeps
            # the chain alive through DCE.
            wz = cpool.tile([128, M_BLK], bf16)
            nc.gpsimd.memset(wz[:], 0.0)
            pewarm = pe_pool.tile([128, M_BLK], fp32, tag="epsum",
                                  name="pewarm")
            for i in range(10):
                nc.tensor.matmul(pewarm[:], wz[:, 0:128], wz[:],
                                 start=(i == 0), stop=(i == 9))
            warm_sb = cpool.tile([1, 1], fp32)
            nc.vector.tensor_copy(warm_sb[:], pewarm[0:1, 0:1])
            warm_dram = dpool.tile([1, 1], fp32)
            nc.sync.dma_start(warm_dram[:], warm_sb[:])

            def softmax_range(p0, p1):
                """Softmax over 64-wide batch groups for partitions
                [p0, p1) of the [128, 2, B] regrouped score view."""
                PP = p1 - p0
                sc2 = mpool.tile([PP, 2, B], fp32, tag="sc2",
                                 name=f"sc2_{p0}")
                src = sc_dram.rearrange("o (p g c) -> (o p) g c", p=128, g=2)
                nc.sync.dma_start(sc2[:], src[p0:p1])
                probs = mpool.tile([PP, 2, B], fp32, tag="probs",
                                   name=f"probs_{p0}")
                sums = mpool.tile([PP, 2], fp32, tag="sums",
                                  name=f"sums_{p0}")
                for g in range(2):
                    nc.scalar.activation(probs[:, g, :], sc2[:, g, :], AF.Exp,
                                         accum_out=sums[:, g:g + 1])
                rsum = mpool.tile([PP, 2], fp32, tag="rsum",
                                  name=f"rsum_{p0}")
                nc.vector.reciprocal(rsum[:], sums[:])
                for g in range(2):
                    nc.vector.tensor_scalar_mul(probs[:, g, :], probs[:, g, :],
                                                rsum[:, g:g + 1])
                dst = out.rearrange("(p g) c -> p g c", g=2)
                nc.sync.dma_start(dst[p0:p1], probs[:])

            def score_block(et, mb, m0, blk, tag):
                """Energy GEMM + tanh + *v + h-sum + partition-reduce for
                tokens [m0, m0+blk); et chunk c holds d-tiles 2c, 2c+1 of
                the block starting at mb*M_BLK (m0 offset within it)."""
                off = m0 - mb * M_BLK
                acc = apool.tile([128, blk], fp32, tag="acc",
                                 name=f"acc{tag}")
                prev_veng = None
                for ht in range(H_TILES):
                    pe = pe_pool.tile([128, blk], fp32, tag="epsum")
                    for dt in range(D_TILES):
                        nc.tensor.matmul(
                            pe[:], wt_sb[ht][:, dt, :],
                            et[dt // 2][:, dt % 2, off:off + blk],
                            start=(dt == 0), stop=(dt == D_TILES - 1))
                    eng = gpool.tile([128, blk], fp32, tag="eng")
                    nc.scalar.activation(eng[:], pe[:], AF.Tanh,
                                         bias=b_sb[:, ht:ht + 1])
                    veng = vpool.tile([128, blk], fp32, tag="veng",
                                      name=f"veng{tag}_{ht}")
                    nc.scalar.mul(veng[:], eng[:], v_sb[:, ht:ht + 1])
                    # running accumulation: ready ~one ACT after the last MM
                    if ht == 1:
                        nc.vector.tensor_add(acc[:], prev_veng[:], veng[:])
                    elif ht > 1:
                        nc.vector.tensor_add(acc[:], acc[:], veng[:])
                    prev_veng = veng
                # scores[m] = sum over all 1024 h = partition-reduce of acc
                red = apool.tile([128, blk], fp32, tag="red",
                                 name=f"red{tag}")
                nc.gpsimd.partition_all_reduce(red[:], acc[:], 128,
                                               bass_isa.ReduceOp.add)
                nc.sync.dma_start(sc_dram[:, m0:m0 + blk], red[0:1, :])

            v_bf = cpool.tile([128, H_TILES], bf16)
            nc.vector.tensor_copy(v_bf[:], v_sb[:])

            def tail_block(et, mb, m0, blk):
                """Last tokens: scores via M=1 bf16 matmuls (deferred one
                h-tile so the PE never waits on ScalarE) and an inline
                single-partition softmax — a much shorter critical chain
                than the gpsimd/DRAM-bounce path."""
                off = m0 - mb * M_BLK
                nl = blk // B  # l rows covered
                sps = ps1pool.tile([1, blk], fp32, tag="sps")
                engs = []
                for ht in range(H_TILES):
                    pe = pe_pool.tile([128, blk], fp32, tag="epsum")
                    for dt in range(D_TILES):
                        nc.tensor.matmul(
                            pe[:], wt_sb[ht][:, dt, :],
                            et[dt // 2][:, dt % 2, off:off + blk],
                            start=(dt == 0), stop=(dt == D_TILES - 1))
                    eng = gpool.tile([128, blk], bf16, tag="engbf",
                                     name=f"engbf{ht}")
                    nc.scalar.activation(eng[:], pe[:], AF.Tanh,
                                         bias=b_sb[:, ht:ht + 1])
                    engs.append(eng)
                    # defer the score matvec two h-tiles so it never waits
                    # on the ScalarE queue
                    if ht >= 2:
                        nc.tensor.matmul(sps[:], v_bf[:, ht - 2:ht - 1],
                                         engs[ht - 2][:], start=(ht == 2),
                                         stop=False)
                for ht in (H_TILES - 2, H_TILES - 1):
                    nc.tensor.matmul(sps[:], v_bf[:, ht:ht + 1],
                                     engs[ht][:], start=False,
                                     stop=(ht == H_TILES - 1))
                st = mpool.tile([1, nl, B], fp32, tag="st")
                nc.scalar.activation(st[:], sps.rearrange("o (l c) -> o l c",
                                                          c=B), AF.Exp)
                tsum = mpool.tile([1, nl], fp32, tag="tsum")
                nc.vector.reduce_sum(tsum[:], st[:],
                                     axis=mybir.AxisListType.X)
                trs = mpool.tile([1, nl], fp32, tag="trs")
                nc.vector.reciprocal(trs[:], tsum[:])
                nc.vector.tensor_tensor(st[:], st[:],
                                        trs[:, :, None].to_broadcast(st.shape),
                                        mybir.AluOpType.mult)
                l0 = m0 // B
                dst = out.rearrange("(a l) c -> a l c", l=nl)
                nc.sync.dma_start(dst[l0 // nl:l0 // nl + 1], st[:])

            TAIL = 256  # tokens on the short-chain path (l 252..255)
            for mb in range(N_BLKS):
                et = et0 if mb == 0 else load_et(mb)
                if mb == N_BLKS - 1:
                    score_block(et, mb, mb * M_BLK, M_BLK - TAIL,
                                f"{mb}_0")
                    # everything except the tail rows can normalize now
                    softmax_range(96, 128 - TAIL // 128)
                    tail_block(et, mb, mb * M_BLK + (M_BLK - TAIL), TAIL)
                else:
                    score_block(et, mb, mb * M_BLK, M_BLK, str(mb))
                if mb == 7:
                    softmax_range(0, 32)
                elif mb == 15:
                    softmax_range(32, 64)
                elif mb == 23:
                    softmax_range(64, 96)

    nc.compile()
    return nc


def kernel(num_features, encoder_outputs, W, b, v):
    global LAST_RESULTS
    from concourse.bass_utils import run_bass_kernel_spmd

    enc = np.asarray(encoder_outputs, dtype=np.float32)
    W_np = np.asarray(W, dtype=np.float32)
    b_np = np.asarray(b, dtype=np.float32)
    v_np = np.asarray(v, dtype=np.float32)
    F = int(np.asarray(num_features))
    assert enc.shape == (L, B, D) and W_np.shape == (H, D)

    # wr[ht, d, j] = W[ht*128 + j, d]
    wr_np = np.ascontiguousarray(
        W_np.reshape(H_TILES, 128, D).transpose(0, 2, 1)).astype(BF16)
    bT_np = np.ascontiguousarray(b_np.reshape(H_TILES, 128).T)     # [128, 8]
    vT_np = np.ascontiguousarray(v_np.ravel().reshape(H_TILES, 128).T)

    in_maps = []
    for c in range(N_CORES):
        shard = enc[c * L_LOC:(c + 1) * L_LOC].reshape(M, D).astype(BF16)
        encT_np = np.ascontiguousarray(shard.T)                    # [D, M]
        in_maps.append({"encT": encT_np, "wr": wr_np, "bT": bT_np,
                        "vT": vT_np})

    if "nc" not in _compiled:
        _compiled["nc"] = _build()
    nc = _compiled["nc"]

    res = run_bass_kernel_spmd(nc, in_maps, core_ids=list(range(N_CORES)))
    LAST_RESULTS = res

    probs = np.concatenate([res.results[c]["out"] for c in range(N_CORES)],
                           axis=0)                                 # [L, B]
    out = np.broadcast_to(probs.T[:, None, :], (B, F, L))
    return np.ascontiguousarray(out)
